# revision 1
# baseline (speedup 1.0000x reference)
"""LoRA MultiheadAttention on 8 Trainium2 NeuronCores (Bass/Tile).

Sharding: core c = (batch n = c//2, head-group hg = c%2); each core handles
6 of 12 heads for one of 4 batches. LoRA is folded into the projection
weights on the host (W_eff = W + scale * up @ down — mathematically
identical). Inputs are shipped pre-transposed (E-major) per shard. Each core
computes q^T/k^T (E-major), v (S-major, with a ones column per head for the
softmax denominator), full-softmax attention in fp16 with fp32 accumulation,
and a half-K out-projection partial. The host sums the two partials per
batch and adds the output bias (pure unshard glue).
"""
import numpy as np

import concourse.bass as bass
import concourse.tile as tile
from concourse import bacc, mybir
from concourse.bass_utils import run_bass_kernel_spmd

L, N, E, H, R = 2048, 4, 768, 12, 16
ALPHA = 16.0
LORA_SCALE = ALPHA / R
HD = E // H          # 64
HG = 2               # head groups (column-parallel dimension)
HPG = H // HG        # 6 heads per group
EG = E // HG         # 384 columns per group
NC_ = 8
F32 = mybir.dt.float32
F16 = mybir.dt.float16
SCALE = 1.0 / float(np.sqrt(HD))  # folded into exp's input scale

_CACHED = {}


def _build():
    nc = bacc.Bacc()
    # per-core external I/O (shapes are per-shard)
    xqT = nc.dram_tensor("xqT", [E, L], F32, kind="ExternalInput")
    xkT = nc.dram_tensor("xkT", [E, L], F32, kind="ExternalInput")
    xvT = nc.dram_tensor("xvT", [E, L], F32, kind="ExternalInput")
    wqT = nc.dram_tensor("wqT", [E, EG], F32, kind="ExternalInput")
    wkT = nc.dram_tensor("wkT", [E, EG], F32, kind="ExternalInput")
    wvT = nc.dram_tensor("wvT", [E, EG], F32, kind="ExternalInput")
    woT = nc.dram_tensor("woT", [EG, E], F32, kind="ExternalInput")
    bq = nc.dram_tensor("bq", [EG], F32, kind="ExternalInput")
    bk = nc.dram_tensor("bk", [EG], F32, kind="ExternalInput")
    bv = nc.dram_tensor("bv", [EG], F32, kind="ExternalInput")
    out = nc.dram_tensor("out", [E, L], F32, kind="ExternalOutput")

    KC = E // 128    # 6 contraction chunks
    EC = EG // 128   # 3 output chunks per projection
    LT = L // 128    # 16 l/s tiles
    VW = HPG * (HD + 1)  # 390: per-head 64 v cols + 1 ones col

    with tile.TileContext(nc) as tc:
        with (
            tc.tile_pool(name="stage", bufs=2) as stage,
            tc.tile_pool(name="big", bufs=16) as big,
            tc.tile_pool(name="persist", bufs=1) as persist,
            tc.tile_pool(name="small", bufs=4) as small,
            tc.tile_pool(name="outsb", bufs=3) as outsb_pool,
            tc.tile_pool(name="psum", bufs=1, space="PSUM") as psum,
        ):
            # ---- constants / weights ----
            w16 = {}
            for pname, wdram in (("q", wqT), ("k", wkT), ("v", wvT)):
                for j in range(KC):
                    w32 = stage.tile([128, 2048], F32, tag="stage", name="w32")
                    nc.sync.dma_start(w32[:, :EG], wdram[j * 128:(j + 1) * 128, :])
                    wt = persist.tile([128, EG], F16, name=f"w16_{pname}{j}")
                    nc.vector.tensor_copy(wt[:], w32[:, :EG])
                    w16[pname, j] = wt
            wo16 = []
            for j in range(EC):
                w32 = stage.tile([128, 2048], F32, tag="stage", name="w32")
                nc.sync.dma_start(w32[:, :E], woT[j * 128:(j + 1) * 128, :])
                wt = persist.tile([128, E], F16, name=f"wo16_{j}")
                nc.vector.tensor_copy(wt[:], w32[:, :E])
                wo16.append(wt)

            bias_t = {}
            for bname, bdram in (("q", bq), ("k", bk)):
                for j in range(EC):
                    bt = persist.tile([128, 1], F32, name=f"b_{bname}{j}")
                    nc.sync.dma_start(bt[:], bdram[j * 128:(j + 1) * 128])
                    bias_t[bname, j] = bt

            # ---- projections ----
            qkT = {}   # ("q"|"k", e-chunk) -> (128, L) f16, E-major
            v_aug = []  # 16 tiles (128, VW) f16, per-head [64 v | 1.0]
            for pname, xdram in (("q", xqT), ("k", xkT), ("v", xvT)):
                x16 = []
                for j in range(KC):
                    x32 = stage.tile([128, 2048], F32, tag="stage", name="x32")
                    nc.sync.dma_start(x32[:], xdram[j * 128:(j + 1) * 128, :])
                    xt = big.tile([128, L], F16, tag="big", name="x16")
                    nc.vector.tensor_copy(xt[:], x32[:])
                    x16.append(xt)
                if pname in ("q", "k"):
                    for e in range(EC):
                        dst = persist.tile([128, L], F16, name=f"{pname}T{e}")
                        qkT[pname, e] = dst
                        for lc in range(2):
                            mm = psum.tile([128, 1024], F32, tag="mm", bufs=3,
                                           name="mm_proj")
                            for half in range(2):
                                o_sl = mm[:, half * 512:(half + 1) * 512]
                                l0 = lc * 1024 + half * 512
                                for kk in range(KC):
                                    nc.tensor.matmul(
                                        o_sl,
                                        w16[pname, kk][:, e * 128:(e + 1) * 128],
                                        x16[kk][:, l0:l0 + 512],
                                        start=(kk == 0), stop=(kk == KC - 1),
                                    )
                            nc.vector.tensor_scalar_add(
                                dst[:, lc * 1024:(lc + 1) * 1024], mm[:],
                                bias_t[pname, e][:],
                            )
                else:
                    for st in range(LT):
                        mm = psum.tile([128, 1024], F32, tag="mm", bufs=3,
                                       name="mm_vproj")
                        for kk in range(KC):
                            nc.tensor.matmul(
                                mm[:, 0:EG],
                                x16[kk][:, st * 128:(st + 1) * 128],
                                w16["v", kk][:],
                                start=(kk == 0), stop=(kk == KC - 1),
                            )
                        vt = persist.tile([128, VW], F16, name=f"v_aug{st}")
                        grp = vt.rearrange("p (h c) -> p h c", c=HD + 1)
                        nc.vector.tensor_copy(
                            grp[:, :, 0:HD],
                            mm[:, 0:EG].rearrange("p (h c) -> p h c", c=HD),
                        )
                        nc.vector.memset(grp[:, :, HD:HD + 1], 1.0)
                        v_aug.append(vt)

            # ---- attention (v-stationary attnV: o^T produced directly) ----
            oT = [persist.tile([128, L], F16, name=f"oT{j}") for j in range(EC)]
            for h in range(HPG):
                et, pb = h // 2, (h % 2) * 64
                qs = qkT["q", et][pb:pb + 64, :]
                ks = qkT["k", et][pb:pb + 64, :]
                attn = []
                for st in range(LT):
                    at = big.tile([128, L], F16, tag="big", name="attn")
                    for lc in range(2):
                        sc = psum.tile([128, 1024], F32, tag="mm", bufs=3,
                                       name="mm_sc")
                        for half in range(2):
                            l0 = lc * 1024 + half * 512
                            nc.tensor.matmul(
                                sc[:, half * 512:(half + 1) * 512],
                                ks[:, st * 128:(st + 1) * 128],
                                qs[:, l0:l0 + 512],
                                start=True, stop=True,
                            )
                        nc.scalar.activation(
                            at[:, lc * 1024:(lc + 1) * 1024], sc[:],
                            mybir.ActivationFunctionType.Exp, scale=SCALE,
                        )
                    attn.append(at)
                # o^T_raw (65, L): rows 0-63 = head output (E-major), row 64
                # = softmax denominator (from the ones column of v_aug)
                oTh = persist.tile([65, L], F16, name="oTh", tag="oTh", bufs=2)
                for lc in range(4):
                    ot = psum.tile([65, 512], F32, tag="ot", bufs=2, name="ot")
                    for st in range(LT):
                        nc.tensor.matmul(
                            ot[:],
                            v_aug[st][:, h * (HD + 1):(h + 1) * (HD + 1)],
                            attn[st][:, lc * 512:(lc + 1) * 512],
                            start=(st == 0), stop=(st == LT - 1),
                        )
                    nc.vector.tensor_copy(
                        oTh[:, lc * 512:(lc + 1) * 512], ot[:])
                rec = small.tile([1, L], F16, tag="rec", bufs=2, name="rec")
                with nc.allow_low_precision("softmax denom reciprocal in f16"):
                    nc.vector.reciprocal(rec[:], oTh[64:65, :])
                rbc = small.tile([64, L], F16, tag="rbc", bufs=2, name="rbc")
                nc.gpsimd.partition_broadcast(rbc[:], rec[:])
                nc.vector.tensor_mul(
                    oT[et][pb:pb + 64, :], oTh[0:64, :], rbc[:])

            # ---- out-projection (out^T = W_o^T-chunks @ o^T) ----
            for lc in range(4):
                for eo in range(6):
                    po = psum.tile([128, 1024], F32, tag="mm", bufs=3,
                                   name="mm_out")
                    for j in range(EC):
                        nc.tensor.matmul(
                            po[:, 0:512],
                            wo16[j][:, eo * 128:(eo + 1) * 128],
                            oT[j][:, lc * 512:(lc + 1) * 512],
                            start=(j == 0), stop=(j == EC - 1),
                        )
                    osb = outsb_pool.tile([128, 512], F32, tag="osb", bufs=4,
                                          name="osb")
                    nc.vector.tensor_copy(osb[:], po[:, 0:512])
                    nc.sync.dma_start(
                        out[eo * 128:(eo + 1) * 128,
                            lc * 512:(lc + 1) * 512], osb[:])
    nc.finalize()
    return nc


def kernel(query, key, value, in_proj_weight, in_proj_bias,
           q_down, q_up, k_down, k_up, v_down, v_up,
           out_proj_weight, out_proj_bias, out_down, out_up):
    if "nc" not in _CACHED:
        _CACHED["nc"] = _build()
    nc = _CACHED["nc"]

    f = np.float32
    # fold LoRA into the projection weights (exact algebraic identity)
    w_eff = {}
    for i, (dn, up) in enumerate(((q_down, q_up), (k_down, k_up),
                                  (v_down, v_up))):
        w = in_proj_weight[i * E:(i + 1) * E].astype(f)
        w_eff[i] = w + LORA_SCALE * (up.astype(f) @ dn.astype(f))
    wo_eff = out_proj_weight.astype(f) + LORA_SCALE * (
        out_up.astype(f) @ out_down.astype(f))

    in_maps = []
    for c in range(NC_):
        n, hg = c // 2, c % 2
        sl = slice(hg * EG, (hg + 1) * EG)
        m = {
            "xqT": np.ascontiguousarray(query[:, n, :].T, dtype=f),
            "xkT": np.ascontiguousarray(key[:, n, :].T, dtype=f),
            "xvT": np.ascontiguousarray(value[:, n, :].T, dtype=f),
            "wqT": np.ascontiguousarray(w_eff[0][sl].T, dtype=f),
            "wkT": np.ascontiguousarray(w_eff[1][sl].T, dtype=f),
            "wvT": np.ascontiguousarray(w_eff[2][sl].T, dtype=f),
            "woT": np.ascontiguousarray(wo_eff[:, sl].T, dtype=f),
            "bq": np.ascontiguousarray(in_proj_bias[0:E][sl], dtype=f),
            "bk": np.ascontiguousarray(in_proj_bias[E:2 * E][sl], dtype=f),
            "bv": np.ascontiguousarray(in_proj_bias[2 * E:3 * E][sl], dtype=f),
        }
        in_maps.append(m)

    _CACHED["in_maps"] = in_maps
    res = run_bass_kernel_spmd(nc, in_maps, list(range(NC_)))
    outp = np.empty((L, N, E), dtype=np.float32)
    bo_total = out_proj_bias.astype(f) + wo_eff @ np.ascontiguousarray(
        in_proj_bias[2 * E:3 * E], dtype=f)
    for n in range(N):
        outp[:, n, :] = (res.results[2 * n]["out"]
                         + res.results[2 * n + 1]["out"]).T + bo_total
    return outp



# revision 6
# speedup vs baseline: 1.3193x; 1.3193x over previous
"""LoRA MultiheadAttention on 8 Trainium2 NeuronCores (Bass/Tile), v2.

Sharding: core c = (batch n = c//2, head-group hg = c%2); each core handles
6 of 12 heads for one of 4 batches. LoRA folded into projection weights on
host (exact). Inputs shipped pre-transposed (E-major) in fp16.

Per-core pipeline:
  - q/k/v projections in f16 (PE), bias via DVE; log2(e) folded into Wq/bq
    so PSUM scores arrive as z = log2e * (q.k).
  - scores: two heads packed concurrently in the PE array (row tiling,
    K=64 at partitions 0-63 / 64-127).
  - exp split between ACT (native exp -> fp8e4m3) and DVE (Schraudolph:
    y = z + 56.04 -> int8, bit-punned as e4m3); both write the same fp8
    attn tiles so attnV sees one uniform dtype.
  - attnV: fp8e4m3 DoubleRow matmuls (K=256/instr), M=128 output =
    64 head dims + 64 replicated softmax denominators (ones columns in v).
  - normalize: DVE reciprocal_approx_fast on the replicated denominator
    rows + tensor_mul -> f16 oT.
  - out-projection f16; partial (half-K) output DMA'd as f16; host sums
    the two partials per batch and adds bias (pure unshard glue).
"""
import numpy as np

import concourse.bass as bass
import concourse.tile as tile
from concourse import bacc, mybir
from concourse.bass_utils import run_bass_kernel_spmd

L, N, E, H, R = 2048, 4, 768, 12, 16
ALPHA = 16.0
LORA_SCALE = ALPHA / R
HD = E // H          # 64
HG = 2               # head groups (column-parallel dimension)
HPG = H // HG        # 6 heads per core
EG = E // HG         # 384 columns per group
NC_ = 8
F32 = mybir.dt.float32
F16 = mybir.dt.float16
F8 = mybir.dt.float8e4
I8 = mybir.dt.int8
SCALE = 1.0 / float(np.sqrt(HD))          # 1/8
C1 = float(np.log2(np.e))                 # folded into Wq, bq on host
ACT_SCALE = float(np.log(2.0) / 8.0)      # exp(z*ACT_SCALE) = e^{score/8}
# Schraudolph bias: 8*(e4m3 bias 7) - 8*0.0573 (centering) + 0.5 (truncate
# -> round). DVE writes int8(z + DVE_BIAS), punned as e4m3 ~= exp.
DVE_BIAS = 56.0 - 8.0 * 0.0573 + 0.5

KC = E // 128        # 6 contraction chunks
LT = L // 128        # 16 s tiles
NPAIR = HPG // 2     # 3 head pairs per core
DR = mybir.MatmulPerfMode.DoubleRow

# exp-unit engine split: slot % 12 < DVE_LIM -> DVE else ACT (~42% DVE)
DVE_LIM = 5
DVE_MOD = 12

_CACHED = {}


def _build():
    nc = bacc.Bacc()
    xqT = nc.dram_tensor("xqT", [E, L], F16, kind="ExternalInput")
    xkT = nc.dram_tensor("xkT", [E, L], F16, kind="ExternalInput")
    xvT = nc.dram_tensor("xvT", [E, L], F16, kind="ExternalInput")
    wqT = nc.dram_tensor("wqT", [E, EG], F16, kind="ExternalInput")
    wkT = nc.dram_tensor("wkT", [E, EG], F16, kind="ExternalInput")
    wvT = nc.dram_tensor("wvT", [E, EG], F16, kind="ExternalInput")
    woT = nc.dram_tensor("woT", [EG, E], F16, kind="ExternalInput")
    bq = nc.dram_tensor("bq", [EG], F32, kind="ExternalInput")
    bk = nc.dram_tensor("bk", [EG], F32, kind="ExternalInput")
    out = nc.dram_tensor("out", [E, L], F16, kind="ExternalOutput")

    with tile.TileContext(nc) as tc:
        with (
            tc.tile_pool(name="persist", bufs=1) as persist,
            tc.tile_pool(name="xring", bufs=6) as xring,
            tc.tile_pool(name="atring", bufs=12) as atring,
            tc.tile_pool(name="small", bufs=2) as small,
            tc.tile_pool(name="psum", bufs=1, space="PSUM") as psum,
        ):
            # ---- weights / biases (DMA'd pre-converted f16) ----
            w16 = {}
            for pname, wdram in (("q", wqT), ("k", wkT), ("v", wvT)):
                for j in range(KC):
                    wt = persist.tile([128, EG], F16, name=f"w16_{pname}{j}")
                    nc.sync.dma_start(wt[:], wdram[j * 128:(j + 1) * 128, :])
                    w16[pname, j] = wt
            wo16 = []
            for j in range(EG // 128):
                wt = persist.tile([128, E], F16, name=f"wo16_{j}")
                nc.sync.dma_start(wt[:], woT[j * 128:(j + 1) * 128, :])
                wo16.append(wt)
            bias_t = {}
            for bname, bdram in (("q", bq), ("k", bk)):
                for j in range(NPAIR):
                    bt = persist.tile([128, 1], F32, name=f"b_{bname}{j}")
                    nc.sync.dma_start(bt[:], bdram[j * 128:(j + 1) * 128])
                    bias_t[bname, j] = bt

            # preload the exp activation table off the critical path
            warm = persist.tile([1, 1], F16, name="warm")
            nc.scalar.activation(warm[:], bias_t["q", 0][0:1, :],
                                 mybir.ActivationFunctionType.Exp,
                                 scale=ACT_SCALE)

            # v layout: [p, sp(8), j(2), h(6), c(128)] e4m3,
            # c = [64 ones | 64 v dims]: the ones rows land the replicated
            # softmax denominator at psum partitions 0-63, where the custom
            # reciprocal op expects base-0 input.
            vmega = persist.tile([128, 8 * 2 * HPG * 128], F8, name="vmega")
            vm = vmega.rearrange("p (sp j h c) -> p sp j h c",
                                 sp=8, j=2, h=HPG)
            for h in range(HPG):
                ones_u32 = vm[:, :, :, h, 0:64].bitcast(mybir.dt.uint32)
                nc.vector.memset(ones_u32, 0x38383838)

            # persistent activations
            qkT = {}
            for pname in ("q", "k"):
                for e in range(NPAIR):
                    qkT[pname, e] = persist.tile(
                        [128, L], F16, name=f"{pname}T{e}")
            oT = [persist.tile([128, L], F16, name=f"oT{j}")
                  for j in range(NPAIR)]

            # ---- x input staging ----
            x16 = {}

            def dma_x(pname, xdram):
                for j in range(KC):
                    xt = xring.tile([128, L], F16, tag="x16", name="x16")
                    nc.sync.dma_start(xt[:], xdram[j * 128:(j + 1) * 128, :])
                    x16[pname, j] = xt

            # ---- q/k projections (upfront; f16, PE + DVE bias) ----
            def proj_qk(pname, e, lh):
                ps = psum.tile([128, 1024], F32, tag="sunit", bufs=3,
                               name="ps_proj")
                for half in range(2):
                    o_sl = ps[:, half * 512:(half + 1) * 512]
                    l0 = lh * 1024 + half * 512
                    for kk in range(KC):
                        nc.tensor.matmul(
                            o_sl,
                            w16[pname, kk][:, e * 128:(e + 1) * 128],
                            x16[pname, kk][:, l0:l0 + 512],
                            start=(kk == 0), stop=(kk == KC - 1),
                        )
                nc.vector.tensor_scalar_add(
                    qkT[pname, e][:, lh * 1024:(lh + 1) * 1024], ps[:],
                    bias_t[pname, e][:])

            dma_x("q", xqT)
            for e in range(NPAIR):
                for lh in range(2):
                    proj_qk("q", e, lh)
            dma_x("k", xkT)
            for e in range(NPAIR):
                for lh in range(2):
                    proj_qk("k", e, lh)
            dma_x("v", xvT)

            # ---- v projection unit (interleaved into pair-0 scores) ----
            def proj_v(st):
                ps = psum.tile([128, 1024], F32, tag="sunit", bufs=3,
                               name="ps_vproj")
                for kk in range(KC):
                    nc.tensor.matmul(
                        ps[:, 0:EG],
                        x16["v", kk][:, st * 128:(st + 1) * 128],
                        w16["v", kk][:],
                        start=(kk == 0), stop=(kk == KC - 1),
                    )
                sp, j = st // 2, st % 2
                nc.vector.tensor_copy(
                    vm[:, sp, j, :, 64:128],
                    ps[:, 0:EG].rearrange("p (h c) -> p h c", c=HD),
                )

            # ---- attention ----
            at_tiles = {}   # (pair, sp) -> tile [128, 8192] e4m3

            def scores_exp(p, st, lc):
                ps = psum.tile([128, 1024], F32, tag="sunit", bufs=3,
                               name="ps_sc")
                for pos in range(2):
                    rows = slice(pos * 64, (pos + 1) * 64)
                    nc.tensor.matmul(
                        ps[:, pos * 512:(pos + 1) * 512],
                        qkT["k", p][rows, st * 128:(st + 1) * 128],
                        qkT["q", p][rows, lc * 512:(lc + 1) * 512],
                        start=True, stop=True,
                    )
                sp, j = st // 2, st % 2
                if (p, sp) not in at_tiles:
                    at_tiles[p, sp] = atring.tile(
                        [128, 2 * 2 * L], F8, tag="at2", name="at2")
                r4 = at_tiles[p, sp].rearrange(
                    "p (j h l) -> p j h l", j=2, h=2)
                out_ap = r4[:, j, :, lc * 512:(lc + 1) * 512]
                slot = (st * 4 + lc) % DVE_MOD
                if slot < DVE_LIM:
                    nc.vector.tensor_scalar_add(
                        out_ap.bitcast(I8), ps[:], DVE_BIAS)
                else:
                    nc.scalar.activation(
                        out_ap, ps[:], mybir.ActivationFunctionType.Exp,
                        scale=ACT_SCALE)

            def attnv_pass(q_pair, pos, half, tag):
                h = q_pair * 2 + pos
                vt = psum.tile([128, 1024], F32, tag=tag,
                               bufs=(1 if tag == "vacc" else 3),
                               name="ps_vt")
                for sp in range(8):
                    r4 = at_tiles[q_pair, sp].rearrange(
                        "p (j h l) -> p j h l", j=2, h=2)
                    for j in range(2):
                        lhsT = vm[:, sp, j, h, :]
                        for lb in range(2):
                            l0 = half * 1024 + lb * 512
                            nc.tensor.matmul(
                                vt[:, lb * 512:(lb + 1) * 512],
                                lhsT,
                                r4[:, j, pos, l0:l0 + 512],
                                start=(sp == 0 and j == 0),
                                stop=(sp == 7 and j == 1),
                            )
                rt = small.tile([64, 1024], F32, tag="rt", bufs=2, name="rt")
                nc.vector.reciprocal_approx_fast(rt[:], vt[0:64, :])
                nc.vector.tensor_mul(
                    oT[q_pair][pos * 64:(pos + 1) * 64,
                               half * 1024:(half + 1) * 1024],
                    vt[64:128, :], rt[:])

            VPASS = [(0, 0), (1, 0), (0, 1), (1, 1)]  # (pos, half)
            for p in range(NPAIR):
                for st in range(LT):
                    for lc in range(4):
                        scores_exp(p, st, lc)
                    if p == 0:
                        proj_v(st)
                    elif st in (1, 3, 5, 7):
                        pos, half = VPASS[(st - 1) // 2]
                        attnv_pass(p - 1, pos, half, "vacc")
            for pos, half in VPASS:
                attnv_pass(NPAIR - 1, pos, half, "sunit")

            # ---- out-projection (out^T partial = W_o^T-chunks @ o^T) ----
            for lh in range(2):
                for eo in range(6):
                    po = psum.tile([128, 1024], F32, tag="sunit", bufs=3,
                                   name="ps_out")
                    for half in range(2):
                        l0 = lh * 1024 + half * 512
                        for j in range(NPAIR):
                            nc.tensor.matmul(
                                po[:, half * 512:(half + 1) * 512],
                                wo16[j][:, eo * 128:(eo + 1) * 128],
                                oT[j][:, l0:l0 + 512],
                                start=(j == 0), stop=(j == NPAIR - 1),
                            )
                    osb = small.tile([128, 1024], F16, tag="osb", bufs=3,
                                     name="osb")
                    if eo % 2 == 0:
                        nc.scalar.copy(osb[:], po[:])
                    else:
                        nc.vector.tensor_copy(osb[:], po[:])
                    nc.sync.dma_start(
                        out[eo * 128:(eo + 1) * 128,
                            lh * 1024:(lh + 1) * 1024], osb[:])
    nc.finalize()
    return nc


def kernel(query, key, value, in_proj_weight, in_proj_bias,
           q_down, q_up, k_down, k_up, v_down, v_up,
           out_proj_weight, out_proj_bias, out_down, out_up):
    if "nc" not in _CACHED:
        _CACHED["nc"] = _build()
    nc = _CACHED["nc"]

    f = np.float32
    h = np.float16
    # fold LoRA into the projection weights (exact algebraic identity)
    w_eff = {}
    for i, (dn, up) in enumerate(((q_down, q_up), (k_down, k_up),
                                  (v_down, v_up))):
        w = in_proj_weight[i * E:(i + 1) * E].astype(f)
        w_eff[i] = w + LORA_SCALE * (up.astype(f) @ dn.astype(f))
    wo_eff = out_proj_weight.astype(f) + LORA_SCALE * (
        out_up.astype(f) @ out_down.astype(f))
    # fold log2(e) into Wq / bq so device scores are log2e * (q.k)
    wq_s = C1 * w_eff[0]
    bq_s = C1 * in_proj_bias[0:E].astype(f)

    in_maps = []
    for c in range(NC_):
        n, hg = c // 2, c % 2
        sl = slice(hg * EG, (hg + 1) * EG)
        m = {
            "xqT": np.ascontiguousarray(query[:, n, :].T, dtype=h),
            "xkT": np.ascontiguousarray(key[:, n, :].T, dtype=h),
            "xvT": np.ascontiguousarray(value[:, n, :].T, dtype=h),
            "wqT": np.ascontiguousarray(wq_s[sl].T, dtype=h),
            "wkT": np.ascontiguousarray(w_eff[1][sl].T, dtype=h),
            "wvT": np.ascontiguousarray(w_eff[2][sl].T, dtype=h),
            "woT": np.ascontiguousarray(wo_eff[:, sl].T, dtype=h),
            "bq": np.ascontiguousarray(bq_s[sl], dtype=f),
            "bk": np.ascontiguousarray(in_proj_bias[E:2 * E][sl], dtype=f),
        }
        in_maps.append(m)

    _CACHED["in_maps"] = in_maps
    res = run_bass_kernel_spmd(nc, in_maps, list(range(NC_)))
    outp = np.empty((L, N, E), dtype=np.float32)
    bo_total = out_proj_bias.astype(f) + wo_eff @ np.ascontiguousarray(
        in_proj_bias[2 * E:3 * E], dtype=f)
    for n in range(N):
        outp[:, n, :] = (res.results[2 * n]["out"].astype(f)
                         + res.results[2 * n + 1]["out"].astype(f)).T + bo_total
    return outp


# revision 11
# speedup vs baseline: 1.6640x; 1.2613x over previous
"""LoRA MultiheadAttention on 8 Trainium2 NeuronCores (Bass/Tile), v5.

Sharding: core c = (batch n = c//2, head-group hg = c%2); each core handles
6 of 12 heads for one of 4 batches. LoRA folded into projection weights on
host (exact). Inputs shipped pre-transposed (E-major) in fp16.

Per-core pipeline (engine-balanced, software-pipelined):
  - q/k projections in f16 (PE, DVE bias); e0 upfront, e1/e2 + the whole v
    projection interleaved into pair-0's attention window. log2(e) folded
    into Wq/bq so PSUM scores arrive as z = log2e * (q.k).
  - scores: two heads packed concurrently in the PE array (row tiling,
    K=64 at partitions 0-63 / 64-127), [h1|h2] 512-col halves of one
    [128,1024] PSUM unit.
  - exp: split ACT (native exp -> fp8e4m3) / DVE (Schraudolph: z + 56.04
    -> int8, bit-punned e4m3) with finely interleaved unit assignment so
    neither engine starves on the 3-deep PSUM ring.
  - attnV: fp8 matmuls, M=128 = 64 replicated ones (denominator) + 64 v
    dims; 16-matmul chains per (head, l-quarter), one chain per 2 score
    s-tiles, double-buffered in 2 single-bank PSUM accumulators.
  - normalize: DVE reciprocal_approx_fast (base-0 denominator rows) +
    tensor_mul -> f16 oT.
  - out-projection f16; half-K partial output DMA'd as f16; host sums the
    two partials per batch and adds bias (pure unshard glue).
"""
import numpy as np

import concourse.bass as bass
import concourse.tile as tile
from concourse import bacc, mybir
from concourse.bass_utils import run_bass_kernel_spmd

L, N, E, H, R = 2048, 4, 768, 12, 16
ALPHA = 16.0
LORA_SCALE = ALPHA / R
HD = E // H          # 64
HG = 2               # head groups (column-parallel dimension)
HPG = H // HG        # 6 heads per core
EG = E // HG         # 384 columns per group
NC_ = 8
F32 = mybir.dt.float32
F16 = mybir.dt.float16
F8 = mybir.dt.float8e4
I8 = mybir.dt.int8
C1 = float(np.log2(np.e))                 # folded into Wq, bq on host
ACT_SCALE = float(np.log(2.0) / 8.0)      # exp(z*ACT_SCALE) = e^{score/8}
# Schraudolph bias: 8*(e4m3 bias 7) - 8*0.0573 (centering) + 0.5 (truncate
# -> round). DVE writes int8(z + DVE_BIAS), punned as e4m3 ~= exp.
DVE_BIAS = 56.0 - 8.0 * 0.0573 + 0.5

KC = E // 128        # 6 contraction chunks
LT = L // 128        # 16 s tiles
NPAIR = HPG // 2     # 3 head pairs per core

_CACHED = {}


def _build():
    nc = bacc.Bacc()
    xqT = nc.dram_tensor("xqT", [E, L], F16, kind="ExternalInput")
    xkT = nc.dram_tensor("xkT", [E, L], F16, kind="ExternalInput")
    xvT = nc.dram_tensor("xvT", [E, L], F16, kind="ExternalInput")
    wqT = nc.dram_tensor("wqT", [E, EG], F16, kind="ExternalInput")
    wkT = nc.dram_tensor("wkT", [E, EG], F16, kind="ExternalInput")
    wvT = nc.dram_tensor("wvT", [E, EG], F16, kind="ExternalInput")
    woT = nc.dram_tensor("woT", [EG, E], F16, kind="ExternalInput")
    bq = nc.dram_tensor("bq", [EG], F32, kind="ExternalInput")
    bk = nc.dram_tensor("bk", [EG], F32, kind="ExternalInput")
    out = nc.dram_tensor("out", [E, L], F16, kind="ExternalOutput")

    with tile.TileContext(nc) as tc:
        with (
            tc.tile_pool(name="persist", bufs=1) as persist,
            tc.tile_pool(name="ring4k", bufs=32) as ring4k,
            tc.tile_pool(name="small", bufs=2) as small,
            tc.tile_pool(name="psum", bufs=1, space="PSUM") as psum,
        ):
            # ---- small persistent state ----
            w16 = {}
            for pname, wdram in (("q", wqT), ("k", wkT), ("v", wvT)):
                for j in range(KC):
                    wt = persist.tile([128, EG], F16, name=f"w16_{pname}{j}")
                    nc.sync.dma_start(wt[:], wdram[j * 128:(j + 1) * 128, :])
                    w16[pname, j] = wt
            wo16 = []
            for j in range(NPAIR):
                wt = persist.tile([128, E], F16, name=f"wo16_{j}")
                nc.sync.dma_start(wt[:], woT[j * 128:(j + 1) * 128, :])
                wo16.append(wt)
            bias_t = {}
            for bname, bdram in (("q", bq), ("k", bk)):
                for j in range(NPAIR):
                    bt = persist.tile([128, 1], F32, name=f"b_{bname}{j}")
                    nc.sync.dma_start(bt[:], bdram[j * 128:(j + 1) * 128])
                    bias_t[bname, j] = bt

            # preload the exp activation table off the critical path
            warm = persist.tile([1, 1], F16, name="warm")
            nc.scalar.activation(warm[:], bias_t["q", 0][0:1, :],
                                 mybir.ActivationFunctionType.Exp,
                                 scale=ACT_SCALE)

            # v layout: [p, st(16), h(6), c(128)] e4m3, c = [64 ones | 64 v]
            # (ones columns land the replicated softmax denominator at psum
            # partitions 0-63, where the custom reciprocal wants base 0).
            vmega = persist.tile([128, LT * HPG * 128], F8, name="vmega")
            vm = vmega.rearrange("p (st h c) -> p st h c", st=LT, h=HPG)
            for h in range(HPG):
                ones_u32 = vm[:, :, h, 0:64].bitcast(mybir.dt.uint32)
                nc.vector.memset(ones_u32, 0x38383838)

            oT = [persist.tile([128, L], F16, name=f"oT{j}")
                  for j in range(NPAIR)]
            qkT = {}
            for pname in ("q", "k"):
                for e in range(NPAIR):
                    qkT[pname, e] = persist.tile(
                        [128, L], F16, name=f"{pname}T{e}")

            # ---- shared 4KB ring: x chunks, qkT, attn tiles ----
            x16 = {}

            def dma_x(pname, xdram):
                for j in range(KC):
                    xt = ring4k.tile([128, L], F16, tag="r4k", name="x16")
                    nc.sync.dma_start(xt[:], xdram[j * 128:(j + 1) * 128, :])
                    x16[pname, j] = xt

            def proj_qk(pname, e, lh, bias_eng):
                ps = psum.tile([128, 1024], F32, tag="sunit", bufs=3,
                               name="ps_proj")
                for half in range(2):
                    o_sl = ps[:, half * 512:(half + 1) * 512]
                    l0 = lh * 1024 + half * 512
                    for kk in range(KC):
                        nc.tensor.matmul(
                            o_sl,
                            w16[pname, kk][:, e * 128:(e + 1) * 128],
                            x16[pname, kk][:, l0:l0 + 512],
                            start=(kk == 0), stop=(kk == KC - 1),
                        )
                dst = qkT[pname, e][:, lh * 1024:(lh + 1) * 1024]
                if bias_eng == "act":
                    nc.scalar.activation(dst, ps[:],
                                         mybir.ActivationFunctionType.Identity,
                                         bias=bias_t[pname, e][:])
                else:
                    nc.vector.tensor_scalar_add(dst, ps[:],
                                                bias_t[pname, e][:])

            def proj_v(st):
                ps = psum.tile([128, 1024], F32, tag="sunit", bufs=3,
                               name="ps_vproj")
                for kk in range(KC):
                    nc.tensor.matmul(
                        ps[:, 0:EG],
                        x16["v", kk][:, st * 128:(st + 1) * 128],
                        w16["v", kk][:],
                        start=(kk == 0), stop=(kk == KC - 1),
                    )
                nc.vector.tensor_copy(
                    vm[:, st, :, 64:128],
                    ps[:, 0:EG].rearrange("p (h c) -> p h c", c=HD),
                )

            # ---- attention building blocks ----
            at_tiles = {}   # (pair, st) -> tile [128, 2, 2048] e4m3

            def scores_exp(p, st, lc, dve):
                ps = psum.tile([128, 1024], F32, tag="sunit", bufs=3,
                               name="ps_sc")
                for pos in range(2):
                    rows = slice(pos * 64, (pos + 1) * 64)
                    nc.tensor.matmul(
                        ps[:, pos * 512:(pos + 1) * 512],
                        qkT["k", p][rows, st * 128:(st + 1) * 128],
                        qkT["q", p][rows, lc * 512:(lc + 1) * 512],
                        start=True, stop=True,
                    )
                if (p, st) not in at_tiles:
                    at_tiles[p, st] = ring4k.tile(
                        [128, 2 * L], F8, tag="r4k", name="at2")
                r3 = at_tiles[p, st].rearrange("p (h l) -> p h l", h=2)
                out_ap = r3[:, :, lc * 512:(lc + 1) * 512]
                if dve:
                    nc.vector.tensor_scalar_add(
                        out_ap.bitcast(I8), ps[:], DVE_BIAS)
                else:
                    nc.scalar.activation(
                        out_ap, ps[:], mybir.ActivationFunctionType.Exp,
                        scale=ACT_SCALE)

            vchain = {}     # chain idx -> psum tile

            def attnv_mms(q_pair, pos, qt, first_half):
                """8 of the 16 accumulating fp8 matmuls for one
                (head, l-quarter) chain."""
                key = (q_pair, pos, qt)
                if first_half:
                    vchain[key] = psum.tile([128, 512], F32, tag="vacc",
                                            bufs=2, name="ps_vt")
                vt = vchain[key]
                h = q_pair * 2 + pos
                l0 = qt * 512
                sts = range(0, 8) if first_half else range(8, 16)
                for st in sts:
                    r3 = at_tiles[q_pair, st].rearrange(
                        "p (h l) -> p h l", h=2)
                    nc.tensor.matmul(
                        vt[:], vm[:, st, h, :], r3[:, pos, l0:l0 + 512],
                        start=(st == 0), stop=(st == LT - 1),
                    )

            def attnv_norm(q_pair, pos, qt):
                vt = vchain.pop((q_pair, pos, qt))
                rt = small.tile([64, 512], F32, tag="rt", bufs=2, name="rt")
                nc.vector.reciprocal_approx_fast(rt[:], vt[0:64, :])
                nc.vector.tensor_mul(
                    oT[q_pair][pos * 64:(pos + 1) * 64,
                               qt * 512:(qt + 1) * 512],
                    vt[64:128, :], rt[:])

            def outproj(eo, lh, copy_eng):
                po = psum.tile([128, 1024], F32, tag="sunit", bufs=3,
                               name="ps_out")
                for half in range(2):
                    l0 = lh * 1024 + half * 512
                    for j in range(NPAIR):
                        nc.tensor.matmul(
                            po[:, half * 512:(half + 1) * 512],
                            wo16[j][:, eo * 128:(eo + 1) * 128],
                            oT[j][:, l0:l0 + 512],
                            start=(j == 0), stop=(j == NPAIR - 1),
                        )
                osb = small.tile([128, 1024], F16, tag="osb", bufs=3,
                                 name="osb")
                if copy_eng == "act":
                    nc.scalar.copy(osb[:], po[:])
                else:
                    nc.vector.tensor_copy(osb[:], po[:])
                nc.sync.dma_start(
                    out[eo * 128:(eo + 1) * 128,
                        lh * 1024:(lh + 1) * 1024], osb[:])

            # ---- head phase: x DMA + e0 projections ----
            dma_x("q", xqT)
            proj_qk("q", 0, 0, "dve")
            proj_qk("q", 0, 1, "dve")
            dma_x("k", xkT)
            proj_qk("k", 0, 0, "dve")
            proj_qk("k", 0, 1, "dve")
            dma_x("v", xvT)

            # remaining projection units interleaved into pair-0 window
            p0_extra = [("q", 1, 0), ("q", 1, 1), ("k", 1, 0), ("k", 1, 1),
                        ("q", 2, 0), ("q", 2, 1), ("k", 2, 0), ("k", 2, 1)]

            # attnV chain order: l-half-major for the tail pair so the
            # out-projection of each l-half can start as soon as possible
            CHAINS = [(0, 0), (1, 0), (0, 1), (1, 1),
                      (0, 2), (1, 2), (0, 3), (1, 3)]

            # ---- main attention loop ----
            for p in range(NPAIR):
                for st in range(LT):
                    for lc in range(4):
                        slot = st * 4 + lc
                        if p == 0:
                            dve = (slot % 3 == 1)
                        else:
                            dve = (slot % 5) in (1, 3)
                        scores_exp(p, st, lc, dve)
                    if p == 0:
                        proj_v(st)
                        if st < 8:
                            pname, e, lh = p0_extra[st]
                            proj_qk(pname, e, lh,
                                    "act" if st % 2 == 0 else "dve")
                    else:
                        c = st // 2
                        if st % 2 == 0 and c >= 2:
                            # free chain c-2's accumulator before chain c
                            # claims its PSUM slot (vacc ring of 2)
                            attnv_norm(p - 1, *CHAINS[c - 2])
                        attnv_mms(p - 1, CHAINS[c][0], CHAINS[c][1],
                                  first_half=(st % 2 == 0))
                if p > 0:
                    for c in range(6, 8):
                        attnv_norm(p - 1, *CHAINS[c])

            # ---- tail: last pair's attnV + out-projection ----
            for ci, (pos, qt) in enumerate(CHAINS):
                attnv_mms(NPAIR - 1, pos, qt, True)
                attnv_mms(NPAIR - 1, pos, qt, False)
                if ci >= 2:
                    attnv_norm(NPAIR - 1, *CHAINS[ci - 2])
                if ci == 5:
                    for eo in range(6):
                        outproj(eo, 0, "act" if eo % 2 == 0 else "dve")
            attnv_norm(NPAIR - 1, *CHAINS[6])
            attnv_norm(NPAIR - 1, *CHAINS[7])
            for eo in range(6):
                outproj(eo, 1, "act" if eo % 2 == 0 else "dve")
    nc.finalize()
    return nc


def kernel(query, key, value, in_proj_weight, in_proj_bias,
           q_down, q_up, k_down, k_up, v_down, v_up,
           out_proj_weight, out_proj_bias, out_down, out_up):
    if "nc" not in _CACHED:
        _CACHED["nc"] = _build()
    nc = _CACHED["nc"]

    f = np.float32
    h = np.float16
    # fold LoRA into the projection weights (exact algebraic identity)
    w_eff = {}
    for i, (dn, up) in enumerate(((q_down, q_up), (k_down, k_up),
                                  (v_down, v_up))):
        w = in_proj_weight[i * E:(i + 1) * E].astype(f)
        w_eff[i] = w + LORA_SCALE * (up.astype(f) @ dn.astype(f))
    wo_eff = out_proj_weight.astype(f) + LORA_SCALE * (
        out_up.astype(f) @ out_down.astype(f))
    # fold log2(e) into Wq / bq so device scores are log2e * (q.k)
    wq_s = C1 * w_eff[0]
    bq_s = C1 * in_proj_bias[0:E].astype(f)

    in_maps = []
    for c in range(NC_):
        n, hg = c // 2, c % 2
        sl = slice(hg * EG, (hg + 1) * EG)
        m = {
            "xqT": np.ascontiguousarray(query[:, n, :].T, dtype=h),
            "xkT": np.ascontiguousarray(key[:, n, :].T, dtype=h),
            "xvT": np.ascontiguousarray(value[:, n, :].T, dtype=h),
            "wqT": np.ascontiguousarray(wq_s[sl].T, dtype=h),
            "wkT": np.ascontiguousarray(w_eff[1][sl].T, dtype=h),
            "wvT": np.ascontiguousarray(w_eff[2][sl].T, dtype=h),
            "woT": np.ascontiguousarray(wo_eff[:, sl].T, dtype=h),
            "bq": np.ascontiguousarray(bq_s[sl], dtype=f),
            "bk": np.ascontiguousarray(in_proj_bias[E:2 * E][sl], dtype=f),
        }
        in_maps.append(m)

    _CACHED["in_maps"] = in_maps
    res = run_bass_kernel_spmd(nc, in_maps, list(range(NC_)))
    outp = np.empty((L, N, E), dtype=np.float32)
    bo_total = out_proj_bias.astype(f) + wo_eff @ np.ascontiguousarray(
        in_proj_bias[2 * E:3 * E], dtype=f)
    for n in range(N):
        outp[:, n, :] = (res.results[2 * n]["out"].astype(f)
                         + res.results[2 * n + 1]["out"].astype(f)).T + bo_total
    return outp
